# revision 3
# baseline (speedup 1.0000x reference)
"""Data-parallel linear layer (x @ W.T + bias) on 8 TRN2 NeuronCores.

Shard x over batch: each core computes a (1024 x 2048) @ (2048 x 2048).T
matmul with bf16 inputs (fp32 PSUM accumulate), bias added on DVE, bf16
outputs cast back to fp32 on host.  bf16 halves HBM traffic so the
kernel is cleanly PE-bound: 512 matmuls x 512 cols @ 2.4 GHz ~= 109 us.

Schedule per core:
 - warmup: 6 matmuls on a memset tile right after the NEFF preamble so
   the PE HAM clock-gate reaches 8/8 (2.4 GHz) by the time real data
   lands.
 - n=0: k-major (PSUM groups for all 8 m interleave per k) -- compute
   starts as soon as the first x k-slab arrives.
 - n=1..3: m-major (16 k-contiguous matmuls per PSUM group) -- drains
   and output DMAs spread evenly, PE never idles at phase boundaries.
 - the very last group (n=3, m=7) is split into two 256-wide chains so
   the final drain+store is half-size and overlaps the second chain.
DMA rings: SP (sync) carries x then w[2]; ACT (scalar) carries w[0],
w[1], bias, w[3], then outputs -- each phase's weights land well before
its first m-chain needs them, and the two final half-stores ride the
otherwise-idle SP ring.
"""
import numpy as np
import ml_dtypes

import concourse.bass as bass  # noqa: F401
import concourse.mybir as mybir
import concourse.tile as tile
from concourse import bacc, bass_utils

B, IN, OUT = 8192, 2048, 2048
NCORES = 8
BS = B // NCORES      # 1024 batch rows per core
P = 128               # partition dim
NFREE = 512           # one PSUM bank of fp32
KT = IN // P          # 16 contraction tiles
MT = BS // P          # 8 output-row tiles per core
NT = OUT // NFREE     # 4 output-col tiles
WARM_MMS = 6          # ~3.2 us of cold-rate PE activity -> HAM 8/8

F32 = mybir.dt.float32
BF16 = mybir.dt.bfloat16
NPBF16 = ml_dtypes.bfloat16

TRACE = False
LAST_EXEC_NS = None

_NC_CACHE = {}


def _build():
    if "nc" in _NC_CACHE:
        return _NC_CACHE["nc"]
    nc = bacc.Bacc("TRN2", target_bir_lowering=False, debug=False)
    xT = nc.dram_tensor("xT", [IN, BS], BF16, kind="ExternalInput")
    wT = nc.dram_tensor("wT", [IN, OUT], BF16, kind="ExternalInput")
    bias_b = nc.dram_tensor("bias_b", [P, OUT], F32, kind="ExternalInput")
    out = nc.dram_tensor("out", [BS, OUT], BF16, kind="ExternalOutput")

    xT_ap = xT.ap()
    wT_ap = wT.ap()
    out_ap = out.ap()

    with tile.TileContext(nc) as tc:
        with tc.tile_pool(name="xp", bufs=KT) as xp, \
             tc.tile_pool(name="wp", bufs=NT * KT) as wp, \
             tc.tile_pool(name="bp", bufs=1) as bp, \
             tc.tile_pool(name="wu", bufs=2) as wu, \
             tc.tile_pool(name="op", bufs=16) as op, \
             tc.tile_pool(name="pp", bufs=8, space="PSUM") as pp:
            bias_sb = bp.tile([P, OUT], F32, tag="bias", name="bias_sb")
            x_sb = [None] * KT
            w_sb = [[None] * KT for _ in range(NT)]

            # warmup operands: no DMA dependency, just DVE memsets
            wu_s = wu.tile([P, P], BF16, tag="wu", name="wu_s")
            wu_m = wu.tile([P, NFREE], BF16, tag="wu", name="wu_m")
            nc.vector.memset(wu_s[:], 0.0)
            nc.vector.memset(wu_m[:], 0.0)

            def emit_x_dma(k):
                t = xp.tile([P, BS], BF16, tag="x", name=f"x_{k}")
                nc.sync.dma_start(t[:], xT_ap[k * P:(k + 1) * P, :])
                x_sb[k] = t

            def emit_w_dma(n, k, eng):
                t = wp.tile([P, NFREE], BF16, tag="w", name=f"w_{n}_{k}")
                eng.dma_start(
                    t[:], wT_ap[k * P:(k + 1) * P,
                                n * NFREE:(n + 1) * NFREE])
                w_sb[n][k] = t

            def mm(n, k, m, ps_m):
                nc.tensor.matmul(
                    ps_m[:],
                    x_sb[k][:, m * P:(m + 1) * P],
                    w_sb[n][k][:],
                    start=(k == 0),
                    stop=(k == KT - 1),
                )

            def drain(n, m, ps_m):
                ot = op.tile([P, NFREE], BF16, tag="o", name=f"o_{n}_{m}")
                nc.vector.tensor_add(
                    ot[:], ps_m[:], bias_sb[:, n * NFREE:(n + 1) * NFREE])
                nc.scalar.dma_start(
                    out_ap[m * P:(m + 1) * P,
                           n * NFREE:(n + 1) * NFREE], ot[:])

            # ring A (sync): x for phase 0, then w[2]
            for k in range(KT):
                emit_x_dma(k)
            for k in range(KT):
                emit_w_dma(2, k, nc.sync)
            # ring B (scalar): w[0] (consumed in lockstep with x), first
            # part of w[1], bias (needed at the first drain), rest of
            # w[1], w[3]; outputs queue behind these
            for k in range(KT):
                emit_w_dma(0, k, nc.scalar)
            for k in range(4):
                emit_w_dma(1, k, nc.scalar)
            nc.scalar.dma_start(bias_sb[:], bias_b.ap())
            for k in range(4, KT):
                emit_w_dma(1, k, nc.scalar)
            for k in range(KT):
                emit_w_dma(3, k, nc.scalar)

            # n=0: k-major, PSUM groups for all 8 m interleave per k
            ps0 = [pp.tile([P, NFREE], F32, tag="ps", name=f"ps_0_{m}")
                   for m in range(MT)]
            for i in range(WARM_MMS):
                nc.tensor.matmul(ps0[0][:], wu_s[:], wu_m[:],
                                 start=True, stop=True)
            for k in range(KT):
                for m in range(MT):
                    mm(0, k, m, ps0[m])
            for m in range(MT):
                drain(0, m, ps0[m])

            # n=1..3: m-major, k-contiguous accumulation chains
            for n in range(1, NT):
                for m in range(MT):
                    if n == NT - 1 and m == MT - 1:
                        break
                    ps_m = pp.tile([P, NFREE], F32, tag="ps",
                                   name=f"ps_{n}_{m}")
                    for k in range(KT):
                        mm(n, k, m, ps_m)
                    drain(n, m, ps_m)

            # last group (n=3, m=7): two half-width chains so the final
            # drain+store is small and overlaps the second chain
            n, m = NT - 1, MT - 1
            for h in range(2):
                ps_h = pp.tile([P, NFREE // 2], F32, tag="ps",
                               name=f"ps_{n}_{m}_{h}")
                for k in range(KT):
                    nc.tensor.matmul(
                        ps_h[:],
                        x_sb[k][:, m * P:(m + 1) * P],
                        w_sb[n][k][:, h * (NFREE // 2):(h + 1) * (NFREE // 2)],
                        start=(k == 0),
                        stop=(k == KT - 1),
                    )
                ot = op.tile([P, NFREE // 2], BF16, tag="o", name=f"o_l{h}")
                noff = n * NFREE + h * (NFREE // 2)
                nc.vector.tensor_add(
                    ot[:], ps_h[:], bias_sb[:, noff:noff + NFREE // 2])
                # final half-stores ride the otherwise-idle SP ring
                nc.sync.dma_start(
                    out_ap[m * P:(m + 1) * P, noff:noff + NFREE // 2], ot[:])
    nc.compile()
    _NC_CACHE["nc"] = nc
    return nc


def kernel(x: np.ndarray, weight: np.ndarray, bias: np.ndarray) -> np.ndarray:
    global LAST_EXEC_NS
    x = np.asarray(x, dtype=np.float32)
    weight = np.asarray(weight, dtype=np.float32)
    bias = np.asarray(bias, dtype=np.float32)

    xT = np.ascontiguousarray(x.T).astype(NPBF16)        # [IN, B]
    wT = np.ascontiguousarray(weight.T).astype(NPBF16)   # [IN, OUT]
    bias_b = np.ascontiguousarray(
        np.broadcast_to(bias[None, :], (P, OUT)), dtype=np.float32)

    in_maps = [
        {
            "xT": np.ascontiguousarray(xT[:, c * BS:(c + 1) * BS]),
            "wT": wT,
            "bias_b": bias_b,
        }
        for c in range(NCORES)
    ]

    nc = _build()
    res = bass_utils.run_bass_kernel_spmd(
        nc, in_maps, core_ids=list(range(NCORES)), trace=TRACE)
    LAST_EXEC_NS = res.exec_time_ns

    return np.concatenate(
        [r["out"].astype(np.float32) for r in res.results], axis=0)


# revision 4
# speedup vs baseline: 1.1397x; 1.1397x over previous
"""Data-parallel linear layer (x @ W.T + bias) on 8 TRN2 NeuronCores.

Shard x over batch: each core computes a (1024 x 2048) @ (2048 x 2048).T
matmul with bf16 inputs (fp32 PSUM accumulate), bias added on DVE, bf16
outputs cast back to fp32 on host.  bf16 halves HBM traffic so the
kernel is cleanly PE-bound: 512 matmuls x 512 cols @ 2.4 GHz ~= 109 us.

Schedule per core:
 - warmup: 6 matmuls on a memset tile right after the NEFF preamble so
   the PE HAM clock-gate reaches 8/8 (2.4 GHz) by the time real data
   lands.
 - n=0: k-major (PSUM groups for all 8 m interleave per k) -- compute
   starts as soon as the first x k-slab arrives.
 - n=1..3: m-major (16 k-contiguous matmuls per PSUM group) -- drains
   and output DMAs spread evenly, PE never idles at phase boundaries.
 - the very last group (n=3, m=7) is split into two 256-wide chains so
   the final drain+store is half-size and overlaps the second chain.
DMA rings: SP (sync) carries x then w[2]; ACT (scalar) carries w[0],
w[1], bias, w[3], then outputs -- each phase's weights land well before
its first m-chain needs them, and the two final half-stores ride the
otherwise-idle SP ring.
"""
import numpy as np
import ml_dtypes

import concourse.bass as bass  # noqa: F401
import concourse.mybir as mybir
import concourse.tile as tile
from concourse import bacc, bass_utils

B, IN, OUT = 8192, 2048, 2048
NCORES = 8
BS = B // NCORES      # 1024 batch rows per core
P = 128               # partition dim
NFREE = 512           # one PSUM bank of fp32
KT = IN // P          # 16 contraction tiles
MT = BS // P          # 8 output-row tiles per core
NT = OUT // NFREE     # 4 output-col tiles
WARM_MMS = 6          # ~3.2 us of cold-rate PE activity -> HAM 8/8

F32 = mybir.dt.float32
BF16 = mybir.dt.bfloat16
NPBF16 = ml_dtypes.bfloat16

TRACE = False
LAST_EXEC_NS = None

_NC_CACHE = {}


def _build():
    if "nc" in _NC_CACHE:
        return _NC_CACHE["nc"]
    nc = bacc.Bacc("TRN2", target_bir_lowering=False, debug=False)
    xT = nc.dram_tensor("xT", [IN, BS], BF16, kind="ExternalInput")
    wT = nc.dram_tensor("wT", [IN, OUT], BF16, kind="ExternalInput")
    bias_b = nc.dram_tensor("bias_b", [P, OUT], F32, kind="ExternalInput")
    out = nc.dram_tensor("out", [BS, OUT], BF16, kind="ExternalOutput")

    xT_ap = xT.ap()
    wT_ap = wT.ap()
    out_ap = out.ap()

    with tile.TileContext(nc) as tc:
        with tc.tile_pool(name="xp", bufs=KT) as xp, \
             tc.tile_pool(name="wp", bufs=NT * KT) as wp, \
             tc.tile_pool(name="bp", bufs=1) as bp, \
             tc.tile_pool(name="wu", bufs=2) as wu, \
             tc.tile_pool(name="op", bufs=16) as op, \
             tc.tile_pool(name="pp", bufs=8, space="PSUM") as pp:
            bias_sb = bp.tile([P, OUT], F32, tag="bias", name="bias_sb")
            x_sb = [None] * KT
            w_sb = [[None] * KT for _ in range(NT)]

            # warmup operands: no DMA dependency, just DVE memsets
            wu_s = wu.tile([P, P], BF16, tag="wu", name="wu_s")
            wu_m = wu.tile([P, NFREE], BF16, tag="wu", name="wu_m")
            nc.vector.memset(wu_s[:], 0.0)
            nc.vector.memset(wu_m[:], 0.0)

            def emit_x_dma(k):
                t = xp.tile([P, BS], BF16, tag="x", name=f"x_{k}")
                nc.sync.dma_start(t[:], xT_ap[k * P:(k + 1) * P, :])
                x_sb[k] = t

            def emit_w_dma(n, k, eng):
                t = wp.tile([P, NFREE], BF16, tag="w", name=f"w_{n}_{k}")
                eng.dma_start(
                    t[:], wT_ap[k * P:(k + 1) * P,
                                n * NFREE:(n + 1) * NFREE])
                w_sb[n][k] = t

            def mm(n, k, m, ps_m):
                nc.tensor.matmul(
                    ps_m[:],
                    x_sb[k][:, m * P:(m + 1) * P],
                    w_sb[n][k][:],
                    start=(k == 0),
                    stop=(k == KT - 1),
                )

            def drain(n, m, ps_m):
                ot = op.tile([P, NFREE], BF16, tag="o", name=f"o_{n}_{m}")
                nc.vector.tensor_add(
                    ot[:], ps_m[:], bias_sb[:, n * NFREE:(n + 1) * NFREE])
                nc.scalar.dma_start(
                    out_ap[m * P:(m + 1) * P,
                           n * NFREE:(n + 1) * NFREE], ot[:])

            # Global DMA emission order MUST match consumption order:
            # the Tile scheduler assigns HWDGE completions to 8 sem
            # lanes round-robin in emission order, and lane counters are
            # monotonic -- a consumer waiting on one DMA transitively
            # waits on every earlier-emitted DMA sharing its lane.
            # Rings: sync carries x + w[2]; scalar carries w[0], bias,
            # w[1], w[3], then outputs.
            for k in range(KT):
                emit_x_dma(k)
                emit_w_dma(0, k, nc.scalar)
            nc.scalar.dma_start(bias_sb[:], bias_b.ap())
            for k in range(KT):
                emit_w_dma(1, k, nc.scalar)
            for k in range(KT):
                emit_w_dma(2, k, nc.sync)
            for k in range(KT):
                emit_w_dma(3, k, nc.scalar)

            # n=0: k-major, PSUM groups for all 8 m interleave per k
            ps0 = [pp.tile([P, NFREE], F32, tag="ps", name=f"ps_0_{m}")
                   for m in range(MT)]
            for i in range(WARM_MMS):
                nc.tensor.matmul(ps0[0][:], wu_s[:], wu_m[:],
                                 start=True, stop=True)
            for k in range(KT):
                for m in range(MT):
                    mm(0, k, m, ps0[m])
            for m in range(MT):
                drain(0, m, ps0[m])

            # n=1..3: m-major, k-contiguous accumulation chains
            for n in range(1, NT):
                for m in range(MT):
                    if n == NT - 1 and m == MT - 1:
                        break
                    ps_m = pp.tile([P, NFREE], F32, tag="ps",
                                   name=f"ps_{n}_{m}")
                    for k in range(KT):
                        mm(n, k, m, ps_m)
                    drain(n, m, ps_m)

            # last group (n=3, m=7): two half-width chains so the final
            # drain+store is small and overlaps the second chain
            n, m = NT - 1, MT - 1
            for h in range(2):
                ps_h = pp.tile([P, NFREE // 2], F32, tag="ps",
                               name=f"ps_{n}_{m}_{h}")
                for k in range(KT):
                    nc.tensor.matmul(
                        ps_h[:],
                        x_sb[k][:, m * P:(m + 1) * P],
                        w_sb[n][k][:, h * (NFREE // 2):(h + 1) * (NFREE // 2)],
                        start=(k == 0),
                        stop=(k == KT - 1),
                    )
                ot = op.tile([P, NFREE // 2], BF16, tag="o", name=f"o_l{h}")
                noff = n * NFREE + h * (NFREE // 2)
                nc.vector.tensor_add(
                    ot[:], ps_h[:], bias_sb[:, noff:noff + NFREE // 2])
                # final half-stores ride the otherwise-idle SP ring
                nc.sync.dma_start(
                    out_ap[m * P:(m + 1) * P, noff:noff + NFREE // 2], ot[:])
    nc.compile()
    _NC_CACHE["nc"] = nc
    return nc


def kernel(x: np.ndarray, weight: np.ndarray, bias: np.ndarray) -> np.ndarray:
    global LAST_EXEC_NS
    x = np.asarray(x, dtype=np.float32)
    weight = np.asarray(weight, dtype=np.float32)
    bias = np.asarray(bias, dtype=np.float32)

    xT = np.ascontiguousarray(x.T).astype(NPBF16)        # [IN, B]
    wT = np.ascontiguousarray(weight.T).astype(NPBF16)   # [IN, OUT]
    bias_b = np.ascontiguousarray(
        np.broadcast_to(bias[None, :], (P, OUT)), dtype=np.float32)

    in_maps = [
        {
            "xT": np.ascontiguousarray(xT[:, c * BS:(c + 1) * BS]),
            "wT": wT,
            "bias_b": bias_b,
        }
        for c in range(NCORES)
    ]

    nc = _build()
    res = bass_utils.run_bass_kernel_spmd(
        nc, in_maps, core_ids=list(range(NCORES)), trace=TRACE)
    LAST_EXEC_NS = res.exec_time_ns

    return np.concatenate(
        [r["out"].astype(np.float32) for r in res.results], axis=0)


# revision 5
# speedup vs baseline: 1.1545x; 1.0130x over previous
"""Data-parallel linear layer (x @ W.T + bias) on 8 TRN2 NeuronCores.

Shard x over batch: each core computes a (1024 x 2048) @ (2048 x 2048).T
matmul with bf16 inputs (fp32 PSUM accumulate), bias added on DVE, bf16
outputs cast back to fp32 on host.  bf16 halves HBM traffic so the
kernel is cleanly PE-bound: 512 matmuls x 512 cols @ 2.4 GHz ~= 109 us.

Schedule per core:
 - warmup: 6 matmuls on a memset tile right after the NEFF preamble so
   the PE HAM clock-gate reaches 8/8 (2.4 GHz) by the time real data
   lands.
 - n=0: k-major (PSUM groups for all 8 m interleave per k) -- compute
   starts as soon as the first x k-slab arrives.
 - n=1..3: m-major (16 k-contiguous matmuls per PSUM group) -- drains
   and output DMAs spread evenly, PE never idles at phase boundaries.
 - the very last group (n=3, m=7) is split into two 256-wide chains so
   the final drain+store is half-size and overlaps the second chain.
DMA rings: SP (sync) carries x then w[2]; ACT (scalar) carries w[0],
w[1], bias, w[3], then outputs -- each phase's weights land well before
its first m-chain needs them, and the two final half-stores ride the
otherwise-idle SP ring.
"""
import numpy as np
import ml_dtypes

import concourse.bass as bass  # noqa: F401
import concourse.mybir as mybir
import concourse.tile as tile
from concourse import bacc, bass_utils

B, IN, OUT = 8192, 2048, 2048
NCORES = 8
BS = B // NCORES      # 1024 batch rows per core
P = 128               # partition dim
NFREE = 512           # one PSUM bank of fp32
KT = IN // P          # 16 contraction tiles
MT = BS // P          # 8 output-row tiles per core
NT = OUT // NFREE     # 4 output-col tiles
WARM_MMS = 6          # ~3.2 us of cold-rate PE activity -> HAM 8/8

F32 = mybir.dt.float32
BF16 = mybir.dt.bfloat16
NPBF16 = ml_dtypes.bfloat16

TRACE = False
LAST_EXEC_NS = None

_NC_CACHE = {}


def _build():
    if "nc" in _NC_CACHE:
        return _NC_CACHE["nc"]
    nc = bacc.Bacc("TRN2", target_bir_lowering=False, debug=False)
    xT = nc.dram_tensor("xT", [IN, BS], BF16, kind="ExternalInput")
    wT = nc.dram_tensor("wT", [IN, OUT], BF16, kind="ExternalInput")
    bias_b = nc.dram_tensor("bias_b", [P, OUT], F32, kind="ExternalInput")
    out = nc.dram_tensor("out", [BS, OUT], BF16, kind="ExternalOutput")

    xT_ap = xT.ap()
    wT_ap = wT.ap()
    out_ap = out.ap()

    with tile.TileContext(nc) as tc:
        with tc.tile_pool(name="xp", bufs=KT) as xp, \
             tc.tile_pool(name="wp", bufs=2 * KT) as wp, \
             tc.tile_pool(name="bp", bufs=1) as bp, \
             tc.tile_pool(name="wu", bufs=2) as wu, \
             tc.tile_pool(name="op", bufs=16) as op, \
             tc.tile_pool(name="pp", bufs=8, space="PSUM") as pp:
            bias_sb = bp.tile([P, OUT], F32, tag="bias", name="bias_sb")
            x_sb = [None] * KT
            w_sb = [[None] * KT for _ in range(NT)]

            # warmup operands: no DMA dependency, just DVE memsets
            wu_s = wu.tile([P, P], BF16, tag="wu", name="wu_s")
            wu_m = wu.tile([P, NFREE], BF16, tag="wu", name="wu_m")
            nc.vector.memset(wu_s[:], 0.0)
            nc.vector.memset(wu_m[:], 0.0)

            def emit_x_dma(k):
                t = xp.tile([P, BS], BF16, tag="x", name=f"x_{k}")
                nc.sync.dma_start(t[:], xT_ap[k * P:(k + 1) * P, :])
                x_sb[k] = t

            def emit_w_dma(n, k, eng):
                t = wp.tile([P, NFREE], BF16, tag="w", name=f"w_{n}_{k}")
                eng.dma_start(
                    t[:], wT_ap[k * P:(k + 1) * P,
                                n * NFREE:(n + 1) * NFREE])
                w_sb[n][k] = t

            def mm(n, k, m, ps_m):
                nc.tensor.matmul(
                    ps_m[:],
                    x_sb[k][:, m * P:(m + 1) * P],
                    w_sb[n][k][:],
                    start=(k == 0),
                    stop=(k == KT - 1),
                )

            def drain(n, m, ps_m):
                ot = op.tile([P, NFREE], BF16, tag="o", name=f"o_{n}_{m}")
                nc.vector.tensor_add(
                    ot[:], ps_m[:], bias_sb[:, n * NFREE:(n + 1) * NFREE])
                nc.scalar.dma_start(
                    out_ap[m * P:(m + 1) * P,
                           n * NFREE:(n + 1) * NFREE], ot[:])

            # Global DMA emission order MUST match consumption order:
            # the Tile scheduler assigns HWDGE completions to 8 sem
            # lanes round-robin in emission order, and lane counters are
            # monotonic -- a consumer waiting on one DMA transitively
            # waits on every earlier-emitted DMA sharing its lane.
            # Rings: sync carries x + w[2]; scalar carries w[0], bias,
            # w[1], w[3], then outputs.
            for k in range(KT):
                emit_x_dma(k)
                emit_w_dma(0, k, nc.scalar)
            nc.scalar.dma_start(bias_sb[:], bias_b.ap())
            for k in range(KT):
                emit_w_dma(1, k, nc.scalar)
            # w[2]/w[3] tiles recycle w[0]/w[1] buffers (wp bufs=2*KT):
            # the WAR dependency paces these fetches to consumption so
            # they cannot starve the x stream during n=0
            for k in range(KT):
                emit_w_dma(2, k, nc.sync)
            for k in range(KT):
                emit_w_dma(3, k, nc.sync)

            # n=0: k-major, PSUM groups for all 8 m interleave per k
            ps0 = [pp.tile([P, NFREE], F32, tag="ps", name=f"ps_0_{m}")
                   for m in range(MT)]
            for i in range(WARM_MMS):
                nc.tensor.matmul(ps0[0][:], wu_s[:], wu_m[:],
                                 start=True, stop=True)
            for k in range(KT):
                for m in range(MT):
                    mm(0, k, m, ps0[m])
            for m in range(MT):
                drain(0, m, ps0[m])

            # n=1..3: m-major, k-contiguous accumulation chains
            for n in range(1, NT):
                for m in range(MT):
                    if n == NT - 1 and m == MT - 1:
                        break
                    ps_m = pp.tile([P, NFREE], F32, tag="ps",
                                   name=f"ps_{n}_{m}")
                    for k in range(KT):
                        mm(n, k, m, ps_m)
                    drain(n, m, ps_m)

            # last group (n=3, m=7): two half-width chains so the final
            # drain+store is small and overlaps the second chain
            n, m = NT - 1, MT - 1
            for h in range(2):
                ps_h = pp.tile([P, NFREE // 2], F32, tag="ps",
                               name=f"ps_{n}_{m}_{h}")
                for k in range(KT):
                    nc.tensor.matmul(
                        ps_h[:],
                        x_sb[k][:, m * P:(m + 1) * P],
                        w_sb[n][k][:, h * (NFREE // 2):(h + 1) * (NFREE // 2)],
                        start=(k == 0),
                        stop=(k == KT - 1),
                    )
                ot = op.tile([P, NFREE // 2], BF16, tag="o", name=f"o_l{h}")
                noff = n * NFREE + h * (NFREE // 2)
                nc.vector.tensor_add(
                    ot[:], ps_h[:], bias_sb[:, noff:noff + NFREE // 2])
                # final half-stores ride the otherwise-idle SP ring
                nc.sync.dma_start(
                    out_ap[m * P:(m + 1) * P, noff:noff + NFREE // 2], ot[:])
    nc.compile()
    _NC_CACHE["nc"] = nc
    return nc


def kernel(x: np.ndarray, weight: np.ndarray, bias: np.ndarray) -> np.ndarray:
    global LAST_EXEC_NS
    x = np.asarray(x, dtype=np.float32)
    weight = np.asarray(weight, dtype=np.float32)
    bias = np.asarray(bias, dtype=np.float32)

    xT = np.ascontiguousarray(x.T).astype(NPBF16)        # [IN, B]
    wT = np.ascontiguousarray(weight.T).astype(NPBF16)   # [IN, OUT]
    bias_b = np.ascontiguousarray(
        np.broadcast_to(bias[None, :], (P, OUT)), dtype=np.float32)

    in_maps = [
        {
            "xT": np.ascontiguousarray(xT[:, c * BS:(c + 1) * BS]),
            "wT": wT,
            "bias_b": bias_b,
        }
        for c in range(NCORES)
    ]

    nc = _build()
    res = bass_utils.run_bass_kernel_spmd(
        nc, in_maps, core_ids=list(range(NCORES)), trace=TRACE)
    LAST_EXEC_NS = res.exec_time_ns

    return np.concatenate(
        [r["out"].astype(np.float32) for r in res.results], axis=0)
